# revision 34
# baseline (speedup 1.0000x reference)
"""Causal self-attention (B=4, T=2048, C=1024, H=16, D=64) on 8 trn2 NeuronCores.

Sharding: core c = (batch b = c//2, head-group g = c%2). Megatron-style within a
batch: each core computes 8 heads' q/k/v (column-parallel) and a row-parallel
partial out-projection. Host sums the two partials per batch and adds the
rank-1 bias term (bo + bv @ wo) -- valid because softmax rows sum to 1, so v's
bias never needs to enter the kernel.

Per-core kernel (all matmuls bf16, fp32 PSUM accumulation):
  The QKV projections (phase 1, chunks 1-3) and the out-projection (phase 3,
  q-chunks 0-2) are decomposed into single-matmul "filler" closures that are
  woven between the attention S^T and AV matmuls, keeping the PE dense while
  the softmax Exp (Scalar) and mask-mult (Vector) chains catch up. PSUM:
  st ring 2x2 banks, O^T accumulators 2x1, filler groups 2x1 = 8 banks.

  phase 1 (per 512-wide T chunk): qT,kT = (x@w)^T via lhsT=w, rhs=x^T (host
           pre-transposes x); v natural via lhsT=x^T-chunk, rhs=wv; a ones
           column is appended to each head's v block. q eviction (scale+bias)
           runs on ScalarE (Identity shares the Exp act table), k/v on Vector.
  phase 2: flash-style streaming attention in S^T orientation:
           S^T[k,q] = kT.T @ qT (head pairs packed in PE row groups 0/64),
           P^T = exp(S^T) (ScalarE, 1/sqrt(D) folded into q), causal masking
           by mask-multiply on diagonal tiles only; O^T accumulated via
           lhsT=v_tile (stationary), rhs=P^T; the ones column of v makes
           PSUM row 64 the softmax denominator Z for free. 1/Z = reciprocal
           on Vector, broadcast across partitions via a DRAM round-trip
           (partition-step-0 DMA reads are legal from DRAM).
  phase 3: y = O @ wo via lhsT=O^T (already the natural layout), rhs=wo;
           eviction copy + store DMA on GpSimd. The last q-chunk's phase 3
           interleaves with its normalization per 128-column slice.
"""
import collections

import numpy as np
import ml_dtypes

import concourse.tile as tile
from concourse import bacc, mybir
from concourse.bass_utils import run_bass_kernel_spmd

BF16 = ml_dtypes.bfloat16
F32 = mybir.dt.float32
BT16 = mybir.dt.bfloat16
AF = mybir.ActivationFunctionType
ALU = mybir.AluOpType

B, T, C, H, D = 4, 2048, 1024, 16, 64
G = 2              # head groups (cores per batch)
HL = H // G        # heads per core = 8
HD = HL * D        # local head dims = 512
NP = 4             # head pairs per core
NJQ = T // 512     # q chunks of 512 = 4
NIK = T // 128     # k tiles of 128 = 16
KC = C // 128      # contraction chunks = 8

_CACHED = {}


def _build():
    nc = bacc.Bacc("TRN2", debug=False)
    xT = nc.dram_tensor("xT", [C, T], BT16, kind="ExternalInput").ap()
    wq = nc.dram_tensor("wq", [C, HD], BT16, kind="ExternalInput").ap()
    wk = nc.dram_tensor("wk", [C, HD], BT16, kind="ExternalInput").ap()
    wv = nc.dram_tensor("wv", [C, HD], BT16, kind="ExternalInput").ap()
    wo = nc.dram_tensor("wo", [HD, C], BT16, kind="ExternalInput").ap()
    bq = nc.dram_tensor("bq", [128, NP], F32, kind="ExternalInput").ap()
    bk = nc.dram_tensor("bk", [128, NP], F32, kind="ExternalInput").ap()
    masks = nc.dram_tensor("masks", [128, 4, 512], BT16, kind="ExternalInput").ap()
    rcp_dram = nc.dram_tensor("rcp_dram", [NJQ, 8, 512], BT16).ap()
    # y stored n-major ([2, T, 512]) so each 128-row store is one contiguous
    # 128KB block (strided [T,C] writes measured ~60GB/s; contiguous ~6x)
    y = nc.dram_tensor("y", [2, T, 512], BT16, kind="ExternalOutput").ap()

    with tile.TileContext(nc) as tc:
        with (
            tc.tile_pool(name="consts", bufs=1) as consts,
            tc.tile_pool(name="xt", bufs=3) as xtp,
            tc.tile_pool(name="qk", bufs=1) as qkp,
            tc.tile_pool(name="vp", bufs=1) as vp,
            tc.tile_pool(name="otp", bufs=1) as otp,
            tc.tile_pool(name="pt", bufs=6) as ptp,
            tc.tile_pool(name="ptmp", bufs=3) as ptmpp,
            tc.tile_pool(name="zn", bufs=3) as znp,
            tc.tile_pool(name="yst", bufs=6) as ystp,
            tc.tile_pool(name="ps", bufs=2, space="PSUM") as ps,
        ):
            # ---- constants. Startup is HBM-bandwidth bound (~330GB/s): order
            # DMAs by first use, finest chunks first, and defer wo (needed at
            # ~t+100us) out of the critical stream. sync/scalar/gpsimd feed
            # three separate DMA queue groups. ----
            # critical stream (xt0, wq, wk) balanced one-per-queue-group;
            # each group delivers only ~1/3 of the ~330GB/s aggregate
            xt0 = xtp.tile([128, KC, 512], BT16, tag="xt", name="xt_pre0")
            x0_r = xT[:, 0:512].rearrange("(k p) t -> p k t", p=128)
            nc.sync.dma_start(xt0[:, 0:2, :], x0_r[:, 0:2, :])
            nc.sync.dma_start(xt0[:, 2:4, :], x0_r[:, 2:4, :])
            nc.sync.dma_start(xt0[:, 4:8, :], x0_r[:, 4:8, :])
            wq_sb = consts.tile([128, KC, HD], BT16, tag="wq")
            wq_r = wq.rearrange("(k p) c -> p k c", p=128)
            nc.scalar.dma_start(wq_sb[:, 0:2, :], wq_r[:, 0:2, :])
            nc.scalar.dma_start(wq_sb[:, 2:4, :], wq_r[:, 2:4, :])
            nc.scalar.dma_start(wq_sb[:, 4:8, :], wq_r[:, 4:8, :])
            wk_sb = consts.tile([128, KC, HD], BT16, tag="wk")
            wk_r = wk.rearrange("(k p) c -> p k c", p=128)
            nc.gpsimd.dma_start(wk_sb[:, 0:4, :], wk_r[:, 0:4, :])
            nc.gpsimd.dma_start(wk_sb[:, 4:8, :], wk_r[:, 4:8, :])
            bq_dma = consts.tile([128, NP], F32, tag="bq_dma")
            bq_sb = consts.tile([128, NP], F32, tag="bq")
            nc.sync.dma_start(bq_dma, bq)
            bk_dma = consts.tile([128, NP], F32, tag="bk_dma")
            bk_sb = consts.tile([128, NP], F32, tag="bk")
            nc.sync.dma_start(bk_dma, bk)
            nc.vector.tensor_copy(bq_sb, bq_dma)
            nc.vector.tensor_copy(bk_sb, bk_dma)
            wv_sb = consts.tile([128, KC, HD], BT16, tag="wv")
            nc.gpsimd.dma_start(wv_sb, wv.rearrange("(k p) c -> p k c", p=128))
            masks_sb = consts.tile([128, 4, 512], BT16, tag="masks")
            nc.scalar.dma_start(masks_sb, masks)
            wo_sb = consts.tile([128, NP, C], BT16, tag="wo")
            ones_sb = consts.tile([1, 64], BT16, tag="ones")
            nc.vector.memset(ones_sb, 1.0)
            # ---- persistent activations ----
            qT = [qkp.tile([128, T], BT16, tag=f"qT{t}", name=f"qT{t}") for t in range(NP)]
            kT = [qkp.tile([128, T], BT16, tag=f"kT{t}", name=f"kT{t}") for t in range(NP)]
            v_sb = [vp.tile([128, HL * 65], BT16, tag=f"v{i}", name=f"v{i}") for i in range(NIK)]
            oT = [otp.tile([128, T], BT16, tag=f"oT{t}", name=f"oT{t}") for t in range(NP)]
            # the ones columns of v never change: set them once here
            for i in range(NIK):
                vg = v_sb[i].rearrange("p (h c) -> p h c", c=65)
                nc.vector.memset(vg[:, :, 64:65], 1.0)

            # ================= filler machinery =================
            # Each filler item is a closure emitting ONE ~213ns PE matmul (and,
            # on group boundaries, the PSUM-group eviction on Scalar/Vector/
            # GpSimd). Consumed between attention S^T and AV matmuls.
            filler = collections.deque()
            consumed = [0]
            p1_watermark = {}   # jt -> total enqueued count that must be consumed
            xt_tiles = {0: xt0}

            def emit_filler(n):
                k = 0
                while k < n and filler:
                    filler.popleft()()
                    consumed[0] += 1
                    k += 1

            def drain_to(watermark):
                while consumed[0] < watermark and filler:
                    filler.popleft()()
                    consumed[0] += 1

            def xt_dma(jt):
                xt = xtp.tile([128, KC, 512], BT16, tag="xt", name=f"xt{jt}")
                xr = xT[:, jt * 512:(jt + 1) * 512].rearrange("(k p) t -> p k t", p=128)
                nc.sync.dma_start(xt[:, 0:4, :], xr[:, 0:4, :])
                nc.gpsimd.dma_start(xt[:, 4:8, :], xr[:, 4:8, :])
                xt_tiles[jt] = xt

            def make_group(nmm, mm_fn, evict_fn, shape, nm):
                st = {}
                def mk(i):
                    def run():
                        if i == 0:
                            st["p"] = ps.tile(shape, F32, tag="fp", name=f"fp_{nm}")
                        mm_fn(st["p"], i, i == 0, i == nmm - 1)
                        if i == nmm - 1:
                            evict_fn(st["p"])
                    return run
                return [mk(i) for i in range(nmm)]

            def enqueue_phase1(jt):
                xt = xt_tiles[jt]
                for t in range(NP):
                    def q_mm(p, k, first, last, t=t, xt=xt):
                        nc.tensor.matmul(
                            p, wq_sb[:, k, t * 128:(t + 1) * 128], xt[:, k, :],
                            start=first, stop=last,
                        )
                    def q_ev(p, t=t, jt=jt):
                        nc.scalar.activation(
                            qT[t][:, jt * 512:(jt + 1) * 512], p,
                            AF.Identity, bias=bq_sb[:, t:t + 1], scale=0.125,
                        )
                    filler.extend(make_group(KC, q_mm, q_ev, [128, 512], f"q{jt}_{t}"))

                    def k_mm(p, k, first, last, t=t, xt=xt):
                        nc.tensor.matmul(
                            p, wk_sb[:, k, t * 128:(t + 1) * 128], xt[:, k, :],
                            start=first, stop=last,
                        )
                    def k_ev(p, t=t, jt=jt):
                        nc.vector.tensor_scalar_add(
                            kT[t][:, jt * 512:(jt + 1) * 512], p, bk_sb[:, t:t + 1]
                        )
                    filler.extend(make_group(KC, k_mm, k_ev, [128, 512], f"k{jt}_{t}"))
                for s in range(4):
                    ik = jt * 4 + s
                    def v_mm(p, k, first, last, s=s, xt=xt):
                        nc.tensor.matmul(
                            p, xt[:, k, s * 128:(s + 1) * 128], wv_sb[:, k, :],
                            start=first, stop=last,
                        )
                    def v_ev(p, ik=ik):
                        vg = v_sb[ik].rearrange("p (h c) -> p h c", c=65)
                        nc.vector.tensor_copy(
                            vg[:, :, 0:64], p.rearrange("p (h c) -> p h c", c=64)
                        )
                    filler.extend(make_group(KC, v_mm, v_ev, [128, 512], f"v{ik}"))
                p1_watermark[jt] = consumed[0] + len(filler)

            def phase3_m(m, via_filler):
                for n in range(2):
                    def y_mm(p, t, first, last, m=m, n=n):
                        nc.tensor.matmul(
                            p, oT[t][:, m * 128:(m + 1) * 128],
                            wo_sb[:, t, n * 512:(n + 1) * 512],
                            start=first, stop=last,
                        )
                    def y_ev(p, m=m, n=n):
                        ys = ystp.tile([128, 512], BT16, tag="y", name=f"ys{m}_{n}")
                        nc.scalar.copy(ys, p)
                        eng = [nc.gpsimd, nc.sync, nc.scalar][(2 * m + n) % 3]
                        eng.dma_start(y[n, m * 128:(m + 1) * 128, :], ys)
                    g = make_group(NP, y_mm, y_ev, [128, 512], f"y{m}_{n}")
                    if via_filler:
                        filler.extend(g)
                    else:
                        for c in g:
                            c()

            def enqueue_phase3(jq):
                for m in range(4 * jq, 4 * jq + 4):
                    phase3_m(m, via_filler=True)

            # ================= attention =================
            def av(t, ik, nik, pts, o_ps):
                pt, c0 = pts[ik]
                ptg = pt.rearrange("p (h q) -> p h q", q=512)
                for hh in range(2):
                    h = 2 * t + hh
                    nc.tensor.matmul(
                        o_ps[hh][:, c0:512], v_sb[ik][:, h * 65:h * 65 + 65],
                        ptg[:, hh, c0:512],
                        start=(ik == 0), stop=(ik == nik - 1),
                    )

            import concourse.bass as bass_mod

            def normalize_start(t, jq, evicted):
                # evicted: [(ouz_h0, ...), (ouz_h1, ...)] for pair t at chunk jq.
                # Pack both heads' Z rows [1,512] as [8,64] each -> one [16,64]
                # reciprocal (64 elems/lane), then broadcast 1/Z via a DRAM
                # round-trip (partition-step-0 DMA reads are legal from DRAM).
                zb = znp.tile([16, 64], F32, tag="zb", bufs=2, name=f"zb{t}_{jq}")
                for hh in range(2):
                    ouz = evicted[hh]
                    nc.sync.dma_start(
                        zb[8 * hh:8 * hh + 8, :],
                        ouz[64:65, :].rearrange("o (p q) -> o p q", p=8),
                    )
                rcp = znp.tile([16, 64], F32, tag="rcpb", bufs=2, name=f"rcp{t}_{jq}")
                nc.vector.reciprocal(rcp, zb)
                rcp16 = znp.tile([16, 64], BT16, tag="rcp16b", bufs=2, name=f"rcp16{t}_{jq}")
                nc.vector.tensor_copy(rcp16, rcp)
                for hh in range(2):
                    nc.sync.dma_start(
                        rcp_dram[jq, 2 * t + hh, :].rearrange("(p q) -> p q", p=8),
                        rcp16[8 * hh:8 * hh + 8, :],
                    )
                tmps = []
                for hh in range(2):
                    ouz = evicted[hh]
                    bc_sb = znp.tile([64, 512], BT16, tag="bc_sb", bufs=3, name=f"bs{t}_{jq}_{hh}")
                    src = rcp_dram[jq, 2 * t + hh, :]
                    bcast = bass_mod.AP(
                        tensor=src.tensor, offset=src.offset,
                        ap=[[0, 64]] + [list(a) for a in src.ap],
                    )
                    nc.sync.dma_start(bc_sb, bcast)
                    tmps.append((ouz, bc_sb))
                return tmps

            def normalize_finish(t, jq, tmps):
                # emitted several tile-steps after normalize_start so the
                # muls never idle-block Vector (k/v evictions queue behind)
                qs2 = slice(jq * 512, (jq + 1) * 512)
                for hh, (ouz, bc_sb) in enumerate(tmps):
                    if hh == 0:
                        nc.vector.tensor_mul(oT[t][0:64, qs2], ouz[0:64, :], bc_sb)
                    else:
                        tmp = znp.tile([64, 512], BT16, tag="tmp_o", bufs=2, name=f"tm{t}_{jq}")
                        nc.vector.tensor_mul(tmp, ouz[0:64, :], bc_sb)
                        nc.gpsimd.dma_start(oT[t][64:128, qs2], tmp)

            def normalize_fast(t, jq, o_ps):
                # Tail-only path for the very last pair: 1/Z straight off the
                # PSUM Z row, partition-broadcast via a K=1 ones outer-product
                # on the (idle) PE -- skips the ouz copy and DRAM round-trip.
                bcs = []
                for hh in range(2):
                    # 1/Z = Exp(-Ln(Z)) on ScalarE (~0.6us/op, same act table);
                    # vector.reciprocal on a [1,512] row costs ~3.3us
                    lnz = znp.tile([1, 512], F32, tag="lnz", bufs=2, name=f"lnz_{hh}")
                    nc.scalar.activation(lnz, o_ps[hh][64:65, :], AF.Ln)
                    rcp1 = znp.tile([1, 512], BT16, tag="rcp1", bufs=2, name=f"rcp1_{hh}")
                    nc.scalar.activation(rcp1, lnz, AF.Exp, scale=-1.0)
                    bc_ps = ps.tile([64, 512], F32, tag="fp", name=f"bcps{hh}")
                    nc.tensor.matmul(bc_ps, ones_sb, rcp1, start=True, stop=True)
                    bc16 = znp.tile([64, 512], BT16, tag="bc_sb", bufs=3, name=f"bc16_{hh}")
                    nc.vector.tensor_copy(bc16, bc_ps)
                    bcs.append(bc16)
                for mi in range(4):
                    cs = slice(mi * 128, (mi + 1) * 128)
                    gs = slice(jq * 512 + mi * 128, jq * 512 + (mi + 1) * 128)
                    for hh, bc16 in enumerate(bcs):
                        if hh == 0:
                            nc.vector.tensor_mul(oT[t][0:64, gs], o_ps[hh][0:64, cs], bc16[:, cs])
                        else:
                            tmp = znp.tile([64, 128], BT16, tag="tmp_os", bufs=4, name=f"tmf{t}_{jq}_{mi}")
                            nc.vector.tensor_mul(tmp, o_ps[hh][0:64, cs], bc16[:, cs])
                            nc.sync.dma_start(oT[t][64:128, gs], tmp)
                    phase3_m(4 * jq + mi, via_filler=False)

            # state for rationing filler across the 160 attention tile-steps
            steps_total = NP * sum(4 * jq + 4 for jq in range(NJQ))
            step_idx = [0]
            pend = []           # (t, jq, evicted) awaiting normalize_start
            pend_fin = []       # (t, jq, tmps) awaiting normalize_finish

            def future_filler(jq):
                # phase3 units for chunks < 3 not yet enqueued
                return 32 * max(0, min(3, NJQ - 1) - jq)

            def attention_sched(t, jq, last=False):
                nik = 4 * jq + 4
                o_ps = [
                    ps.tile([65, 512], F32, tag="ot", bufs=2, name=f"ops{t}_{jq}_{_h}")
                    for _h in range(2)
                ]
                pts = {}
                for ik in range(nik):
                    d = ik - 4 * jq
                    c0 = 128 * d if d > 0 else 0   # first potentially-valid column
                    st = ps.tile([128, 1024], F32, tag="st", name=f"st{t}_{jq}_{ik}")
                    stg = st.rearrange("p (h q) -> p h q", q=512)
                    for hh in range(2):
                        r = slice(hh * 64, hh * 64 + 64)
                        nc.tensor.matmul(
                            stg[:, hh, c0:512],
                            kT[t][r, ik * 128:(ik + 1) * 128],
                            qT[t][r, jq * 512 + c0:(jq + 1) * 512],
                            start=True, stop=True,
                        )
                    pt = ptp.tile([128, 1024], BT16, tag="pt", name=f"pt{t}_{jq}_{ik}")
                    ptg = pt.rearrange("p (h q) -> p h q", q=512)
                    if d >= 0:
                        ptm = ptmpp.tile([128, 1024], BT16, tag="ptmp", name=f"ptm{t}_{jq}_{ik}")
                        ptmg = ptm.rearrange("p (h q) -> p h q", q=512)
                        nc.scalar.activation(ptmg[:, :, c0:512], stg[:, :, c0:512], AF.Exp)
                        for hh in range(2):
                            nc.vector.tensor_mul(
                                ptg[:, hh, c0:512],
                                ptmg[:, hh, c0:512],
                                masks_sb[:, d, c0:512],
                            )
                    else:
                        nc.scalar.activation(pt, st, AF.Exp)
                    pts[ik] = (pt, c0)
                    # ration filler between S^T and the lagging AV; hold back
                    # for the last chunk so its longer attention rows still
                    # get PE cover while Scalar exps catch up
                    steps_left = steps_total - step_idx[0]
                    avail = len(filler) + future_filler(jq)
                    cap = 2 if jq < NJQ - 1 else 4
                    n = min(cap, max(1, -(-avail // max(1, steps_left))))
                    emit_filler(n)
                    step_idx[0] += 1
                    if ik > 0:
                        av(t, ik - 1, nik, pts, o_ps)
                    if ik == 1 and pend:
                        st_, sjq_, sev_ = pend.pop(0)
                        pend_fin.append((st_, sjq_, normalize_start(st_, sjq_, sev_)))
                    if ik == min(nik - 1, 6) and pend_fin:
                        ft_, fjq_, ftm_ = pend_fin.pop(0)
                        normalize_finish(ft_, fjq_, ftm_)
                        if ft_ == NP - 1 and fjq_ < NJQ - 1:
                            enqueue_phase3(fjq_)
                av(t, nik - 1, nik, pts, o_ps)
                if last:
                    # tail pair: normalize_fast reads O^T/Z straight from PSUM
                    pend.append((t, jq, o_ps))
                    return
                # evict Z row + unnormalized O^T, freeing the PSUM accumulators
                out_h = []
                for hh in range(2):
                    ouz = znp.tile([65, 512], F32, tag="ouz", bufs=6, name=f"oz{t}_{jq}_{hh}")
                    nc.vector.tensor_copy(ouz, o_ps[hh])
                    out_h.append(ouz)
                pend.append((t, jq, out_h))

            # ================= main schedule =================
            # upfront: phase 1 chunk 0, k-major with 4 simultaneously-open
            # PSUM groups (2 "fp" + 2 borrowed "ot" slots) so PE consumption
            # tracks the HBM-bound weight stream chunk by chunk instead of
            # each group stalling on the full tensor
            def boot_ps(idx, nm):
                return ps.tile([128, 512], F32, tag=("fp" if idx < 2 else "ot"),
                               name=nm)

            bq_ps = [boot_ps(t, f"boot_q{t}") for t in range(NP)]
            for k in range(KC):
                for t in range(NP):
                    nc.tensor.matmul(
                        bq_ps[t], wq_sb[:, k, t * 128:(t + 1) * 128], xt0[:, k, :],
                        start=(k == 0), stop=(k == KC - 1),
                    )
            for t in range(NP):
                nc.scalar.activation(
                    qT[t][:, 0:512], bq_ps[t],
                    AF.Identity, bias=bq_sb[:, t:t + 1], scale=0.125,
                )
            bk_ps = [boot_ps(t, f"boot_k{t}") for t in range(NP)]
            for k in range(KC):
                for t in range(NP):
                    nc.tensor.matmul(
                        bk_ps[t], wk_sb[:, k, t * 128:(t + 1) * 128], xt0[:, k, :],
                        start=(k == 0), stop=(k == KC - 1),
                    )
            for t in range(NP):
                nc.scalar.activation(
                    kT[t][:, 0:512], bk_ps[t],
                    AF.Identity, bias=bk_sb[:, t:t + 1], scale=1.0,
                )
            bv_ps = [boot_ps(s, f"boot_v{s}") for s in range(4)]
            for k in range(KC):
                for s in range(4):
                    nc.tensor.matmul(
                        bv_ps[s], xt0[:, k, s * 128:(s + 1) * 128], wv_sb[:, k, :],
                        start=(k == 0), stop=(k == KC - 1),
                    )
            for s in range(4):
                vg = v_sb[s].rearrange("p (h c) -> p h c", c=65)
                nc.vector.tensor_copy(
                    vg[:, :, 0:64], bv_ps[s].rearrange("p (h c) -> p h c", c=64)
                )
            # wo is first needed by phase3 at ~t+100us; issue it only now so
            # its 2MB doesn't compete with the startup-critical stream
            nc.gpsimd.dma_start(wo_sb, wo.rearrange("(t p) c -> p t c", p=128))
            # prefetch DMA + enqueue filler for later chunks
            xt_dma(1)
            enqueue_phase1(1)
            for jq in range(NJQ):
                if jq + 2 < NJQ:
                    xt_dma(jq + 2)
                    enqueue_phase1(jq + 2)
                for t in range(NP):
                    attention_sched(t, jq, last=(jq == NJQ - 1 and t == NP - 1))
                if jq + 1 < NJQ:
                    drain_to(p1_watermark[jq + 1])
            # tail: flush stragglers, then the last pair interleaves its
            # normalization with phase3 m-chunks (jq=3's phase3 runs here).
            while len(filler):
                emit_filler(len(filler))
            for ft_, fjq_, ftm_ in pend_fin:
                normalize_finish(ft_, fjq_, ftm_)
                if ft_ == NP - 1 and fjq_ < NJQ - 1:
                    enqueue_phase3(fjq_)
                    emit_filler(len(filler))
            for st_, sjq_, sev_ in pend[:-1]:
                tm_ = normalize_start(st_, sjq_, sev_)
                normalize_finish(st_, sjq_, tm_)
                if st_ == NP - 1 and sjq_ < NJQ - 1:
                    enqueue_phase3(sjq_)
                    emit_filler(len(filler))
            pt_, pjq_, pev_ = pend[-1]
            normalize_fast(pt_, pjq_, pev_)

    nc.compile()
    return nc


def _host_prep(x, wq, bq, wk, bk, wv, wo):
    masks_np = np.zeros((128, 4, 512), dtype=BF16)
    qn = np.arange(512)[None, :]
    kn = np.arange(128)[:, None]
    for d in range(4):
        masks_np[:, d, :] = (qn >= kn + 128 * d).astype(BF16)

    per_g = []
    for g in range(G):
        cs = slice(g * HD, (g + 1) * HD)
        per_g.append({
            "wq": np.ascontiguousarray(wq[:, cs]).astype(BF16),
            "wk": np.ascontiguousarray(wk[:, cs]).astype(BF16),
            "wv": np.ascontiguousarray(wv[:, cs]).astype(BF16),
            "wo": np.ascontiguousarray(wo[cs, :]).astype(BF16),
            "bq": np.ascontiguousarray((bq[cs] / 8.0).reshape(NP, 128).T).astype(np.float32),
            "bk": np.ascontiguousarray(bk[cs].reshape(NP, 128).T).astype(np.float32),
            "masks": masks_np,
        })
    in_maps = []
    for c in range(8):
        b, g = divmod(c, G)
        m = dict(per_g[g])
        m["xT"] = np.ascontiguousarray(x[b].T).astype(BF16)
        in_maps.append(m)
    return in_maps


def kernel(x, wq, bq, wk, bk, wv, bv, wo, bo):
    x = np.asarray(x, dtype=np.float32)
    wq = np.asarray(wq, dtype=np.float32)
    bq = np.asarray(bq, dtype=np.float32)
    wk = np.asarray(wk, dtype=np.float32)
    bk = np.asarray(bk, dtype=np.float32)
    wv = np.asarray(wv, dtype=np.float32)
    bv = np.asarray(bv, dtype=np.float32)
    wo = np.asarray(wo, dtype=np.float32)
    bo = np.asarray(bo, dtype=np.float32)

    if "nc" not in _CACHED:
        _CACHED["nc"] = _build()
    nc = _CACHED["nc"]

    in_maps = _host_prep(x, wq, bq, wk, bk, wv, wo)
    res = run_bass_kernel_spmd(nc, in_maps, core_ids=list(range(8)))

    const_row = (bo.astype(np.float64) + bv.astype(np.float64) @ wo.astype(np.float64))
    out = np.empty((B, T, C), dtype=np.float32)
    for b in range(B):
        # y is stored n-major [2, T, 512]; reassemble the C axis
        y0 = np.asarray(res.results[2 * b]["y"], dtype=np.float64)
        y1 = np.asarray(res.results[2 * b + 1]["y"], dtype=np.float64)
        acc = np.concatenate([y0[0], y0[1]], axis=-1)
        acc += np.concatenate([y1[0], y1[1]], axis=-1)
        acc += const_row[None, :]
        out[b] = acc.astype(np.float32)
    return out


# revision 35
# speedup vs baseline: 1.0356x; 1.0356x over previous
"""Causal self-attention (B=4, T=2048, C=1024, H=16, D=64) on 8 trn2 NeuronCores.

Sharding: core c = (batch b = c//2, head-group g = c%2). Megatron-style within a
batch: each core computes 8 heads' q/k/v (column-parallel) and a row-parallel
partial out-projection. Host sums the two partials per batch and adds the
rank-1 bias term (bo + bv @ wo) -- valid because softmax rows sum to 1, so v's
bias never needs to enter the kernel.

Per-core kernel (all matmuls bf16, fp32 PSUM accumulation):
  phase 1 (per 512-wide T chunk): qT,kT = (x@w)^T via lhsT=w, rhs=x^T (host
           pre-transposes x); v natural via lhsT=x^T-chunk, rhs=wv; a ones
           column is appended to each head's v block.
  phase 2: flash-style streaming attention in S^T orientation:
           S^T[k,q] = kT.T @ qT (head pairs packed in PE row groups 0/64),
           P^T = exp(S^T) (ScalarE, 1/sqrt(D) folded into q), causal masking
           by mask-multiply (GpSimd) on diagonal tiles only; O^T accumulated
           via lhsT=v_tile (stationary), rhs=P^T; the ones column of v makes
           PSUM row 64 the softmax denominator Z for free. 1/Z = Exp(-Log(Z))
           on ScalarE (same activation table set as Exp), broadcast across
           partitions with a K=1 ones outer-product on the PE.
  phase 3 (per T chunk, overlapped with the next chunk's attention):
           y = O @ wo via lhsT=O^T (already the natural layout), rhs=wo.
"""
import numpy as np
import ml_dtypes

import concourse.tile as tile
from concourse import bacc, mybir
from concourse.bass_utils import run_bass_kernel_spmd

BF16 = ml_dtypes.bfloat16
F32 = mybir.dt.float32
BT16 = mybir.dt.bfloat16
AF = mybir.ActivationFunctionType
ALU = mybir.AluOpType

B, T, C, H, D = 4, 2048, 1024, 16, 64
G = 2              # head groups (cores per batch)
HL = H // G        # heads per core = 8
HD = HL * D        # local head dims = 512
NP = 4             # head pairs per core
NJQ = T // 512     # q chunks of 512 = 4
NIK = T // 128     # k tiles of 128 = 16
KC = C // 128      # contraction chunks = 8

_CACHED = {}


def _build():
    nc = bacc.Bacc("TRN2", debug=False)
    xT = nc.dram_tensor("xT", [C, T], BT16, kind="ExternalInput").ap()
    wq = nc.dram_tensor("wq", [C, HD], BT16, kind="ExternalInput").ap()
    wk = nc.dram_tensor("wk", [C, HD], BT16, kind="ExternalInput").ap()
    wv = nc.dram_tensor("wv", [C, HD], BT16, kind="ExternalInput").ap()
    wo = nc.dram_tensor("wo", [HD, C], BT16, kind="ExternalInput").ap()
    bq = nc.dram_tensor("bq", [128, NP], F32, kind="ExternalInput").ap()
    bk = nc.dram_tensor("bk", [128, NP], F32, kind="ExternalInput").ap()
    masks = nc.dram_tensor("masks", [128, 4, 512], BT16, kind="ExternalInput").ap()
    rcp_dram = nc.dram_tensor("rcp_dram", [NJQ, 8, 512], BT16).ap()
    y = nc.dram_tensor("y", [T, C], F32, kind="ExternalOutput").ap()

    with tile.TileContext(nc) as tc:
        with (
            tc.tile_pool(name="consts", bufs=1) as consts,
            tc.tile_pool(name="xt", bufs=3) as xtp,
            tc.tile_pool(name="qk", bufs=1) as qkp,
            tc.tile_pool(name="vp", bufs=1) as vp,
            tc.tile_pool(name="otp", bufs=1) as otp,
            tc.tile_pool(name="pt", bufs=6) as ptp,
            tc.tile_pool(name="ptmp", bufs=3) as ptmpp,
            tc.tile_pool(name="zn", bufs=3) as znp,
            tc.tile_pool(name="yst", bufs=4) as ystp,
            tc.tile_pool(name="ps", bufs=2, space="PSUM") as ps,
        ):
            # ---- constants (biases are tiny and gate evictions: load them first) ----
            bq_dma = consts.tile([128, NP], F32, tag="bq_dma")
            bq_sb = consts.tile([128, NP], F32, tag="bq")
            nc.sync.dma_start(bq_dma, bq)
            nc.vector.tensor_copy(bq_sb, bq_dma)
            bk_dma = consts.tile([128, NP], F32, tag="bk_dma")
            bk_sb = consts.tile([128, NP], F32, tag="bk")
            nc.sync.dma_start(bk_dma, bk)
            nc.vector.tensor_copy(bk_sb, bk_dma)
            wq_sb = consts.tile([128, KC, HD], BT16, tag="wq")
            wq_r = wq.rearrange("(k p) c -> p k c", p=128)
            xt0 = xtp.tile([128, KC, 512], BT16, tag="xt", name="xt_pre0")
            x0_r = xT[:, 0:512].rearrange("(k p) t -> p k t", p=128)
            for k in range(KC):
                nc.scalar.dma_start(wq_sb[:, k, :], wq_r[:, k, :])
                nc.sync.dma_start(xt0[:, k, :], x0_r[:, k, :])
            wk_sb = consts.tile([128, KC, HD], BT16, tag="wk")
            wk_r = wk.rearrange("(k p) c -> p k c", p=128)
            for k in range(KC):
                nc.sync.dma_start(wk_sb[:, k, :], wk_r[:, k, :])
            masks_dma = consts.tile([128, 4, 512], BT16, tag="masks_dma")
            masks_sb = consts.tile([128, 4, 512], BT16, tag="masks")
            nc.gpsimd.dma_start(masks_dma, masks)
            nc.gpsimd.tensor_copy(masks_sb, masks_dma)
            wv_sb = consts.tile([128, KC, HD], BT16, tag="wv")
            nc.gpsimd.dma_start(wv_sb, wv.rearrange("(k p) c -> p k c", p=128))
            wo_sb = consts.tile([128, NP, C], BT16, tag="wo")
            nc.gpsimd.dma_start(wo_sb, wo.rearrange("(t p) c -> p t c", p=128))
            # ---- persistent activations ----
            qT = [qkp.tile([128, T], BT16, tag=f"qT{t}", name=f"qT{t}") for t in range(NP)]
            kT = [qkp.tile([128, T], BT16, tag=f"kT{t}", name=f"kT{t}") for t in range(NP)]
            v_sb = [vp.tile([128, HL * 65], BT16, tag=f"v{i}", name=f"v{i}") for i in range(NIK)]
            oT = [otp.tile([128, T], BT16, tag=f"oT{t}", name=f"oT{t}") for t in range(NP)]

            def phase1(jt):
                if jt == 0:
                    xt = xt0
                else:
                    xt = xtp.tile([128, KC, 512], BT16, tag="xt", name=f"xt{jt}")
                    xr = xT[:, jt * 512:(jt + 1) * 512].rearrange("(k p) t -> p k t", p=128)
                    for k in range(KC):
                        (nc.sync if k % 2 == 0 else nc.scalar).dma_start(xt[:, k, :], xr[:, k, :])
                for t in range(NP):
                    p = ps.tile([128, 512], F32, tag="st", name=f"pq{jt}_{t}")
                    for k in range(KC):
                        nc.tensor.matmul(
                            p, wq_sb[:, k, t * 128:(t + 1) * 128], xt[:, k, :],
                            start=(k == 0), stop=(k == KC - 1),
                        )
                    nc.vector.tensor_scalar(
                        qT[t][:, jt * 512:(jt + 1) * 512], p,
                        0.125, bq_sb[:, t:t + 1], ALU.mult, ALU.add,
                    )
                for t in range(NP):
                    p = ps.tile([128, 512], F32, tag="st", name=f"pk{jt}_{t}")
                    for k in range(KC):
                        nc.tensor.matmul(
                            p, wk_sb[:, k, t * 128:(t + 1) * 128], xt[:, k, :],
                            start=(k == 0), stop=(k == KC - 1),
                        )
                    nc.vector.tensor_scalar_add(
                        kT[t][:, jt * 512:(jt + 1) * 512], p, bk_sb[:, t:t + 1]
                    )
                for s in range(4):
                    ik = jt * 4 + s
                    p = ps.tile([128, 512], F32, tag="st", name=f"pv{ik}")
                    for k in range(KC):
                        nc.tensor.matmul(
                            p, xt[:, k, s * 128:(s + 1) * 128], wv_sb[:, k, :],
                            start=(k == 0), stop=(k == KC - 1),
                        )
                    vg = v_sb[ik].rearrange("p (h c) -> p h c", c=65)
                    nc.vector.tensor_copy(
                        vg[:, :, 0:64], p.rearrange("p (h c) -> p h c", c=64)
                    )
                    nc.vector.memset(vg[:, :, 64:65], 1.0)

            def av(t, ik, nik, pts, o_ps):
                pt, c0 = pts[ik]
                ptg = pt.rearrange("p (h q) -> p h q", q=512)
                for hh in range(2):
                    h = 2 * t + hh
                    nc.tensor.matmul(
                        o_ps[hh][:, c0:512], v_sb[ik][:, h * 65:h * 65 + 65],
                        ptg[:, hh, c0:512],
                        start=(ik == 0), stop=(ik == nik - 1),
                    )

            def attention(t, jq):
                nik = 4 * jq + 4
                qs = slice(jq * 512, (jq + 1) * 512)
                o_ps = [
                    ps.tile([65, 512], F32, tag="ot", bufs=4, name=f"ops{t}_{jq}_{_h}")
                    for _h in range(2)
                ]
                pts = {}
                for ik in range(nik):
                    d = ik - 4 * jq
                    c0 = 128 * d if d > 0 else 0   # first potentially-valid column
                    st = ps.tile([128, 1024], F32, tag="st", name=f"st{t}_{jq}_{ik}")
                    stg = st.rearrange("p (h q) -> p h q", q=512)
                    for hh in range(2):
                        r = slice(hh * 64, hh * 64 + 64)
                        nc.tensor.matmul(
                            stg[:, hh, c0:512],
                            kT[t][r, ik * 128:(ik + 1) * 128],
                            qT[t][r, jq * 512 + c0:(jq + 1) * 512],
                            start=True, stop=True,
                        )
                    pt = ptp.tile([128, 1024], BT16, tag="pt", name=f"pt{t}_{jq}_{ik}")
                    ptg = pt.rearrange("p (h q) -> p h q", q=512)
                    if d >= 0:
                        ptm = ptmpp.tile([128, 1024], BT16, tag="ptmp", name=f"ptm{t}_{jq}_{ik}")
                        ptmg = ptm.rearrange("p (h q) -> p h q", q=512)
                        nc.scalar.activation(ptmg[:, :, c0:512], stg[:, :, c0:512], AF.Exp)
                        for hh in range(2):
                            nc.vector.tensor_mul(
                                ptg[:, hh, c0:512],
                                ptmg[:, hh, c0:512],
                                masks_sb[:, d, c0:512],
                            )
                    else:
                        nc.scalar.activation(pt, st, AF.Exp)
                    pts[ik] = (pt, c0)
                    if ik > 0:
                        av(t, ik - 1, nik, pts, o_ps)
                av(t, nik - 1, nik, pts, o_ps)
                # evict Z row + unnormalized O^T, freeing the PSUM accumulators
                out_h = []
                for hh in range(2):
                    ouz = znp.tile([65, 512], F32, tag="ouz", bufs=6, name=f"oz{t}_{jq}_{hh}")
                    nc.vector.tensor_copy(ouz, o_ps[hh])
                    out_h.append(ouz)
                return out_h

            def phase3_m(m):
                    for n in range(2):
                        p = ps.tile([128, 512], F32, tag="st", name=f"py{m}_{n}")
                        for t in range(NP):
                            nc.tensor.matmul(
                                p, oT[t][:, m * 128:(m + 1) * 128],
                                wo_sb[:, t, n * 512:(n + 1) * 512],
                                start=(t == 0), stop=(t == NP - 1),
                            )
                        ys = ystp.tile([128, 512], F32, tag="y", name=f"ys{m}_{n}")
                        nc.vector.tensor_copy(ys, p)
                        nc.gpsimd.dma_start(
                            y[m * 128:(m + 1) * 128, n * 512:(n + 1) * 512], ys
                        )

            def phase3(jq):
                for m in range(4 * jq, 4 * jq + 4):
                    phase3_m(m)

            import concourse.bass as bass_mod

            def normalize(t, jq, evicted, interleave_phase3=False):
                # evicted: [(ouz_h0, ...), (ouz_h1, ...)] for pair t at chunk jq.
                # Pack both heads' Z rows [1,512] as [8,64] each -> one [16,64]
                # reciprocal (64 elems/lane), then broadcast 1/Z via a DRAM
                # round-trip (partition-step-0 DMA reads are legal from DRAM).
                qs2 = slice(jq * 512, (jq + 1) * 512)
                zb = znp.tile([16, 64], F32, tag="zb", bufs=2, name=f"zb{t}_{jq}")
                for hh in range(2):
                    ouz = evicted[hh]
                    nc.sync.dma_start(
                        zb[8 * hh:8 * hh + 8, :],
                        ouz[64:65, :].rearrange("o (p q) -> o p q", p=8),
                    )
                rcp = znp.tile([16, 64], F32, tag="rcpb", bufs=2, name=f"rcp{t}_{jq}")
                nc.vector.reciprocal(rcp, zb)
                rcp16 = znp.tile([16, 64], BT16, tag="rcp16b", bufs=2, name=f"rcp16{t}_{jq}")
                nc.vector.tensor_copy(rcp16, rcp)
                for hh in range(2):
                    nc.sync.dma_start(
                        rcp_dram[jq, 2 * t + hh, :].rearrange("(p q) -> p q", p=8),
                        rcp16[8 * hh:8 * hh + 8, :],
                    )
                tmps = []
                for hh in range(2):
                    ouz = evicted[hh]
                    bc_sb = znp.tile([64, 512], BT16, tag="bc_sb", bufs=3, name=f"bs{t}_{jq}_{hh}")
                    src = rcp_dram[jq, 2 * t + hh, :]
                    bcast = bass_mod.AP(
                        tensor=src.tensor, offset=src.offset,
                        ap=[[0, 64]] + [list(a) for a in src.ap],
                    )
                    nc.sync.dma_start(bc_sb, bcast)
                    tmps.append((ouz, bc_sb))
                if not interleave_phase3:
                    for hh, (ouz, bc_sb) in enumerate(tmps):
                        if hh == 0:
                            nc.vector.tensor_mul(oT[t][0:64, qs2], ouz[0:64, :], bc_sb)
                        else:
                            tmp = znp.tile([64, 512], BT16, tag="tmp_o", bufs=2, name=f"tm{t}_{jq}")
                            nc.vector.tensor_mul(tmp, ouz[0:64, :], bc_sb)
                            nc.gpsimd.dma_start(oT[t][64:128, qs2], tmp)
                else:
                    # last pair of the last chunk: per-128-col muls, phase3
                    # m-chunk follows immediately after its slice is ready
                    for mi in range(4):
                        cs = slice(mi * 128, (mi + 1) * 128)
                        gs = slice(jq * 512 + mi * 128, jq * 512 + (mi + 1) * 128)
                        for hh, (ouz, bc_sb) in enumerate(tmps):
                            if hh == 0:
                                nc.vector.tensor_mul(oT[t][0:64, gs], ouz[0:64, cs], bc_sb[:, cs])
                            else:
                                tmp = znp.tile([64, 128], BT16, tag="tmp_os", bufs=4, name=f"tms{t}_{jq}_{mi}")
                                nc.vector.tensor_mul(tmp, ouz[0:64, cs], bc_sb[:, cs])
                                nc.sync.dma_start(oT[t][64:128, gs], tmp)
                        phase3_m(4 * jq + mi)

            phase1(0)
            pend = []          # (t, jq, evicted) not yet normalized
            for jq in range(NJQ):
                for t in range(NP):
                    ev = attention(t, jq)
                    if jq == 0 and t == 0 and NJQ > 1:
                        phase1(1)
                    if pend:
                        pt_, pjq_, pev_ = pend.pop(0)
                        normalize(pt_, pjq_, pev_)
                        if pt_ == NP - 1:
                            phase3(pjq_)
                    pend.append((t, jq, ev))
                if jq + 2 < NJQ:
                    phase1(jq + 2)
            # tail: all but the last pending entry normally; the last one
            # interleaves its normalization with phase3 m-chunks
            for pt_, pjq_, pev_ in pend[:-1]:
                normalize(pt_, pjq_, pev_)
                if pt_ == NP - 1:
                    phase3(pjq_)
            pt_, pjq_, pev_ = pend[-1]
            normalize(pt_, pjq_, pev_, interleave_phase3=True)

    nc.compile()
    return nc


def _host_prep(x, wq, bq, wk, bk, wv, wo):
    masks_np = np.zeros((128, 4, 512), dtype=BF16)
    qn = np.arange(512)[None, :]
    kn = np.arange(128)[:, None]
    for d in range(4):
        masks_np[:, d, :] = (qn >= kn + 128 * d).astype(BF16)

    per_g = []
    for g in range(G):
        cs = slice(g * HD, (g + 1) * HD)
        per_g.append({
            "wq": np.ascontiguousarray(wq[:, cs]).astype(BF16),
            "wk": np.ascontiguousarray(wk[:, cs]).astype(BF16),
            "wv": np.ascontiguousarray(wv[:, cs]).astype(BF16),
            "wo": np.ascontiguousarray(wo[cs, :]).astype(BF16),
            "bq": np.ascontiguousarray((bq[cs] / 8.0).reshape(NP, 128).T).astype(np.float32),
            "bk": np.ascontiguousarray(bk[cs].reshape(NP, 128).T).astype(np.float32),
            "masks": masks_np,
        })
    in_maps = []
    for c in range(8):
        b, g = divmod(c, G)
        m = dict(per_g[g])
        m["xT"] = np.ascontiguousarray(x[b].T).astype(BF16)
        in_maps.append(m)
    return in_maps


def kernel(x, wq, bq, wk, bk, wv, bv, wo, bo):
    x = np.asarray(x, dtype=np.float32)
    wq = np.asarray(wq, dtype=np.float32)
    bq = np.asarray(bq, dtype=np.float32)
    wk = np.asarray(wk, dtype=np.float32)
    bk = np.asarray(bk, dtype=np.float32)
    wv = np.asarray(wv, dtype=np.float32)
    bv = np.asarray(bv, dtype=np.float32)
    wo = np.asarray(wo, dtype=np.float32)
    bo = np.asarray(bo, dtype=np.float32)

    if "nc" not in _CACHED:
        _CACHED["nc"] = _build()
    nc = _CACHED["nc"]

    in_maps = _host_prep(x, wq, bq, wk, bk, wv, wo)
    res = run_bass_kernel_spmd(nc, in_maps, core_ids=list(range(8)))

    const_row = (bo.astype(np.float64) + bv.astype(np.float64) @ wo.astype(np.float64))
    out = np.empty((B, T, C), dtype=np.float32)
    for b in range(B):
        acc = res.results[2 * b]["y"].astype(np.float64)
        acc += res.results[2 * b + 1]["y"]
        acc += const_row[None, :]
        out[b] = acc.astype(np.float32)
    return out



# revision 37
# speedup vs baseline: 1.1906x; 1.1496x over previous
"""Causal self-attention (B=4, T=2048, C=1024, H=16, D=64) on 8 trn2 NeuronCores.

Sharding: core c = (batch b = c//2, head-group g = c%2). Megatron-style within a
batch: each core computes 8 heads' q/k/v (column-parallel) and a row-parallel
partial out-projection. Host sums the two partials per batch and adds the
rank-1 bias term (bo + bv @ wo) -- valid because softmax rows sum to 1, so v's
bias never needs to enter the kernel.

Per-core kernel (all matmuls bf16, fp32 PSUM accumulation):
  The QKV projections (phase 1, chunks 1-3) and the out-projection (phase 3,
  q-chunks 0-2) are decomposed into single-matmul "filler" closures that are
  woven between the attention S^T and AV matmuls, keeping the PE dense while
  the softmax Exp (Scalar) and mask-mult (Vector) chains catch up. PSUM:
  st ring 2x2 banks, O^T accumulators 2x1, filler groups 2x1 = 8 banks.

  phase 1 (per 512-wide T chunk): qT,kT = (x@w)^T via lhsT=w, rhs=x^T (host
           pre-transposes x); v natural via lhsT=x^T-chunk, rhs=wv; a ones
           column is appended to each head's v block. q eviction (scale+bias)
           runs on ScalarE (Identity shares the Exp act table), k/v on Vector.
  phase 2: flash-style streaming attention in S^T orientation:
           S^T[k,q] = kT.T @ qT (head pairs packed in PE row groups 0/64),
           P^T = exp(S^T) (ScalarE, 1/sqrt(D) folded into q), causal masking
           by mask-multiply on diagonal tiles only; O^T accumulated via
           lhsT=v_tile (stationary), rhs=P^T; the ones column of v makes
           PSUM row 64 the softmax denominator Z for free. 1/Z = reciprocal
           on Vector, broadcast across partitions via a DRAM round-trip
           (partition-step-0 DMA reads are legal from DRAM).
  phase 3: y = O @ wo via lhsT=O^T (already the natural layout), rhs=wo;
           eviction copy + store DMA on GpSimd. The last q-chunk's phase 3
           interleaves with its normalization per 128-column slice.
"""
import collections

import numpy as np
import ml_dtypes

import concourse.tile as tile
from concourse import bacc, mybir
from concourse.bass_utils import run_bass_kernel_spmd

BF16 = ml_dtypes.bfloat16
F32 = mybir.dt.float32
BT16 = mybir.dt.bfloat16
AF = mybir.ActivationFunctionType
ALU = mybir.AluOpType

B, T, C, H, D = 4, 2048, 1024, 16, 64
G = 2              # head groups (cores per batch)
HL = H // G        # heads per core = 8
HD = HL * D        # local head dims = 512
NP = 4             # head pairs per core
NJQ = T // 512     # q chunks of 512 = 4
NIK = T // 128     # k tiles of 128 = 16
KC = C // 128      # contraction chunks = 8

_CACHED = {}


def _build():
    nc = bacc.Bacc("TRN2", debug=False)
    xT = nc.dram_tensor("xT", [C, T], BT16, kind="ExternalInput").ap()
    wq = nc.dram_tensor("wq", [C, HD], BT16, kind="ExternalInput").ap()
    wk = nc.dram_tensor("wk", [C, HD], BT16, kind="ExternalInput").ap()
    wv = nc.dram_tensor("wv", [C, HD], BT16, kind="ExternalInput").ap()
    wo = nc.dram_tensor("wo", [HD, C], BT16, kind="ExternalInput").ap()
    bq = nc.dram_tensor("bq", [128, NP], F32, kind="ExternalInput").ap()
    bk = nc.dram_tensor("bk", [128, NP], F32, kind="ExternalInput").ap()
    masks = nc.dram_tensor("masks", [128, 4, 512], BT16, kind="ExternalInput").ap()
    rcp_dram = nc.dram_tensor("rcp_dram", [NJQ, 8, 512], BT16).ap()
    # y stored n-major ([2, T, 512]) so each 128-row store is one contiguous
    # 128KB block (strided [T,C] writes measured ~60GB/s; contiguous ~6x)
    y = nc.dram_tensor("y", [2, T, 512], BT16, kind="ExternalOutput").ap()

    with tile.TileContext(nc) as tc:
        with (
            tc.tile_pool(name="consts", bufs=1) as consts,
            tc.tile_pool(name="xt", bufs=3) as xtp,
            tc.tile_pool(name="qk", bufs=1) as qkp,
            tc.tile_pool(name="vp", bufs=1) as vp,
            tc.tile_pool(name="otp", bufs=1) as otp,
            tc.tile_pool(name="pt", bufs=6) as ptp,
            tc.tile_pool(name="ptmp", bufs=3) as ptmpp,
            tc.tile_pool(name="zn", bufs=3) as znp,
            tc.tile_pool(name="yst", bufs=6) as ystp,
            tc.tile_pool(name="ps", bufs=2, space="PSUM") as ps,
        ):
            # ---- constants. Startup is HBM-bandwidth bound (~330GB/s): order
            # DMAs by first use, finest chunks first, and defer wo (needed at
            # ~t+100us) out of the critical stream. sync/scalar/gpsimd feed
            # three separate DMA queue groups. ----
            # critical stream (xt0, wq, wk) balanced one-per-queue-group;
            # each group delivers only ~1/3 of the ~330GB/s aggregate
            xt0 = xtp.tile([128, KC, 512], BT16, tag="xt", name="xt_pre0")
            x0_r = xT[:, 0:512].rearrange("(k p) t -> p k t", p=128)
            nc.sync.dma_start(xt0[:, 0:2, :], x0_r[:, 0:2, :])
            nc.sync.dma_start(xt0[:, 2:4, :], x0_r[:, 2:4, :])
            nc.sync.dma_start(xt0[:, 4:8, :], x0_r[:, 4:8, :])
            wq_sb = consts.tile([128, KC, HD], BT16, tag="wq")
            wq_r = wq.rearrange("(k p) c -> p k c", p=128)
            nc.scalar.dma_start(wq_sb[:, 0:2, :], wq_r[:, 0:2, :])
            nc.scalar.dma_start(wq_sb[:, 2:4, :], wq_r[:, 2:4, :])
            nc.scalar.dma_start(wq_sb[:, 4:8, :], wq_r[:, 4:8, :])
            wk_sb = consts.tile([128, KC, HD], BT16, tag="wk")
            wk_r = wk.rearrange("(k p) c -> p k c", p=128)
            nc.gpsimd.dma_start(wk_sb[:, 0:4, :], wk_r[:, 0:4, :])
            nc.gpsimd.dma_start(wk_sb[:, 4:8, :], wk_r[:, 4:8, :])
            bq_dma = consts.tile([128, NP], F32, tag="bq_dma")
            bq_sb = consts.tile([128, NP], F32, tag="bq")
            nc.sync.dma_start(bq_dma, bq)
            bk_dma = consts.tile([128, NP], F32, tag="bk_dma")
            bk_sb = consts.tile([128, NP], F32, tag="bk")
            nc.sync.dma_start(bk_dma, bk)
            nc.vector.tensor_copy(bq_sb, bq_dma)
            nc.vector.tensor_copy(bk_sb, bk_dma)
            wv_sb = consts.tile([128, KC, HD], BT16, tag="wv")
            nc.gpsimd.dma_start(wv_sb, wv.rearrange("(k p) c -> p k c", p=128))
            masks_sb = consts.tile([128, 4, 512], BT16, tag="masks")
            nc.scalar.dma_start(masks_sb, masks)
            wo_sb = consts.tile([128, NP, C], BT16, tag="wo")
            ones_sb = consts.tile([1, 64], BT16, tag="ones")
            nc.vector.memset(ones_sb, 1.0)
            # ---- persistent activations ----
            qT = [qkp.tile([128, T], BT16, tag=f"qT{t}", name=f"qT{t}") for t in range(NP)]
            kT = [qkp.tile([128, T], BT16, tag=f"kT{t}", name=f"kT{t}") for t in range(NP)]
            v_sb = [vp.tile([128, HL * 65], BT16, tag=f"v{i}", name=f"v{i}") for i in range(NIK)]
            oT = [otp.tile([128, T], BT16, tag=f"oT{t}", name=f"oT{t}") for t in range(NP)]
            # the ones columns of v never change: set them once here
            for i in range(NIK):
                vg = v_sb[i].rearrange("p (h c) -> p h c", c=65)
                nc.vector.memset(vg[:, :, 64:65], 1.0)

            # ================= filler machinery =================
            # Each filler item is a closure emitting ONE ~213ns PE matmul (and,
            # on group boundaries, the PSUM-group eviction on Scalar/Vector/
            # GpSimd). Consumed between attention S^T and AV matmuls.
            filler = collections.deque()
            consumed = [0]
            p1_watermark = {}   # jt -> total enqueued count that must be consumed
            xt_tiles = {0: xt0}

            def emit_filler(n):
                k = 0
                while k < n and filler:
                    filler.popleft()()
                    consumed[0] += 1
                    k += 1

            def drain_to(watermark):
                while consumed[0] < watermark and filler:
                    filler.popleft()()
                    consumed[0] += 1

            def xt_dma(jt):
                xt = xtp.tile([128, KC, 512], BT16, tag="xt", name=f"xt{jt}")
                xr = xT[:, jt * 512:(jt + 1) * 512].rearrange("(k p) t -> p k t", p=128)
                nc.sync.dma_start(xt[:, 0:4, :], xr[:, 0:4, :])
                nc.gpsimd.dma_start(xt[:, 4:8, :], xr[:, 4:8, :])
                xt_tiles[jt] = xt

            def make_group(nmm, mm_fn, evict_fn, shape, nm):
                st = {}
                def mk(i):
                    def run():
                        if i == 0:
                            st["p"] = ps.tile(shape, F32, tag="fp", name=f"fp_{nm}")
                        mm_fn(st["p"], i, i == 0, i == nmm - 1)
                        if i == nmm - 1:
                            evict_fn(st["p"])
                    return run
                return [mk(i) for i in range(nmm)]

            def enqueue_phase1(jt):
                xt = xt_tiles[jt]
                for t in range(NP):
                    def q_mm(p, k, first, last, t=t, xt=xt):
                        nc.tensor.matmul(
                            p, wq_sb[:, k, t * 128:(t + 1) * 128], xt[:, k, :],
                            start=first, stop=last,
                        )
                    def q_ev(p, t=t, jt=jt):
                        nc.scalar.activation(
                            qT[t][:, jt * 512:(jt + 1) * 512], p,
                            AF.Identity, bias=bq_sb[:, t:t + 1], scale=0.125,
                        )
                    filler.extend(make_group(KC, q_mm, q_ev, [128, 512], f"q{jt}_{t}"))

                    def k_mm(p, k, first, last, t=t, xt=xt):
                        nc.tensor.matmul(
                            p, wk_sb[:, k, t * 128:(t + 1) * 128], xt[:, k, :],
                            start=first, stop=last,
                        )
                    def k_ev(p, t=t, jt=jt):
                        nc.vector.tensor_scalar_add(
                            kT[t][:, jt * 512:(jt + 1) * 512], p, bk_sb[:, t:t + 1]
                        )
                    filler.extend(make_group(KC, k_mm, k_ev, [128, 512], f"k{jt}_{t}"))
                for s in range(4):
                    ik = jt * 4 + s
                    def v_mm(p, k, first, last, s=s, xt=xt):
                        nc.tensor.matmul(
                            p, xt[:, k, s * 128:(s + 1) * 128], wv_sb[:, k, :],
                            start=first, stop=last,
                        )
                    def v_ev(p, ik=ik):
                        vg = v_sb[ik].rearrange("p (h c) -> p h c", c=65)
                        nc.vector.tensor_copy(
                            vg[:, :, 0:64], p.rearrange("p (h c) -> p h c", c=64)
                        )
                    filler.extend(make_group(KC, v_mm, v_ev, [128, 512], f"v{ik}"))
                p1_watermark[jt] = consumed[0] + len(filler)

            def phase3_m(m, via_filler):
                for n in range(2):
                    def y_mm(p, t, first, last, m=m, n=n):
                        nc.tensor.matmul(
                            p, oT[t][:, m * 128:(m + 1) * 128],
                            wo_sb[:, t, n * 512:(n + 1) * 512],
                            start=first, stop=last,
                        )
                    def y_ev(p, m=m, n=n):
                        ys = ystp.tile([128, 512], BT16, tag="y", name=f"ys{m}_{n}")
                        nc.scalar.copy(ys, p)
                        eng = nc.gpsimd if (m + n) % 2 == 0 else nc.sync
                        eng.dma_start(y[n, m * 128:(m + 1) * 128, :], ys)
                    g = make_group(NP, y_mm, y_ev, [128, 512], f"y{m}_{n}")
                    if via_filler:
                        filler.extend(g)
                    else:
                        for c in g:
                            c()

            def enqueue_phase3(jq):
                for m in range(4 * jq, 4 * jq + 4):
                    phase3_m(m, via_filler=True)

            # ================= attention =================
            def av(t, ik, nik, pts, o_ps):
                pt, c0 = pts[ik]
                ptg = pt.rearrange("p (h q) -> p h q", q=512)
                for hh in range(2):
                    h = 2 * t + hh
                    nc.tensor.matmul(
                        o_ps[hh][:, c0:512], v_sb[ik][:, h * 65:h * 65 + 65],
                        ptg[:, hh, c0:512],
                        start=(ik == 0), stop=(ik == nik - 1),
                    )

            import concourse.bass as bass_mod

            def normalize_start(t, jq, evicted):
                # evicted: [(ouz_h0, ...), (ouz_h1, ...)] for pair t at chunk jq.
                # Pack both heads' Z rows [1,512] as [8,64] each -> one [16,64]
                # reciprocal (64 elems/lane), then broadcast 1/Z via a DRAM
                # round-trip (partition-step-0 DMA reads are legal from DRAM).
                zb = znp.tile([16, 64], F32, tag="zb", bufs=2, name=f"zb{t}_{jq}")
                for hh in range(2):
                    ouz = evicted[hh]
                    nc.sync.dma_start(
                        zb[8 * hh:8 * hh + 8, :],
                        ouz[64:65, :].rearrange("o (p q) -> o p q", p=8),
                    )
                rcp = znp.tile([16, 64], F32, tag="rcpb", bufs=2, name=f"rcp{t}_{jq}")
                nc.vector.reciprocal(rcp, zb)
                rcp16 = znp.tile([16, 64], BT16, tag="rcp16b", bufs=2, name=f"rcp16{t}_{jq}")
                nc.vector.tensor_copy(rcp16, rcp)
                for hh in range(2):
                    nc.sync.dma_start(
                        rcp_dram[jq, 2 * t + hh, :].rearrange("(p q) -> p q", p=8),
                        rcp16[8 * hh:8 * hh + 8, :],
                    )
                tmps = []
                for hh in range(2):
                    ouz = evicted[hh]
                    bc_sb = znp.tile([64, 512], BT16, tag="bc_sb", bufs=3, name=f"bs{t}_{jq}_{hh}")
                    src = rcp_dram[jq, 2 * t + hh, :]
                    bcast = bass_mod.AP(
                        tensor=src.tensor, offset=src.offset,
                        ap=[[0, 64]] + [list(a) for a in src.ap],
                    )
                    nc.sync.dma_start(bc_sb, bcast)
                    tmps.append((ouz, bc_sb))
                return tmps

            def normalize_finish(t, jq, tmps):
                # emitted several tile-steps after normalize_start so the
                # muls never idle-block Vector (k/v evictions queue behind)
                qs2 = slice(jq * 512, (jq + 1) * 512)
                for hh, (ouz, bc_sb) in enumerate(tmps):
                    if hh == 0:
                        nc.vector.tensor_mul(oT[t][0:64, qs2], ouz[0:64, :], bc_sb)
                    else:
                        tmp = znp.tile([64, 512], BT16, tag="tmp_o", bufs=2, name=f"tm{t}_{jq}")
                        nc.vector.tensor_mul(tmp, ouz[0:64, :], bc_sb)
                        nc.gpsimd.dma_start(oT[t][64:128, qs2], tmp)

            def normalize_fast(t, jq, o_ps):
                # Tail-only path for the very last pair: 1/Z straight off the
                # PSUM Z row, partition-broadcast via a K=1 ones outer-product
                # on the (idle) PE -- skips the ouz copy and DRAM round-trip.
                bcs = []
                for hh in range(2):
                    # 1/Z = Exp(-Ln(Z)) on ScalarE (~0.6us/op, same act table);
                    # vector.reciprocal on a [1,512] row costs ~3.3us
                    lnz = znp.tile([1, 512], F32, tag="lnz", bufs=2, name=f"lnz_{hh}")
                    nc.scalar.activation(lnz, o_ps[hh][64:65, :], AF.Ln)
                    rcp1 = znp.tile([1, 512], BT16, tag="rcp1", bufs=2, name=f"rcp1_{hh}")
                    nc.scalar.activation(rcp1, lnz, AF.Exp, scale=-1.0)
                    bc_ps = ps.tile([64, 512], F32, tag="fp", name=f"bcps{hh}")
                    nc.tensor.matmul(bc_ps, ones_sb, rcp1, start=True, stop=True)
                    bc16 = znp.tile([64, 512], BT16, tag="bc_sb", bufs=3, name=f"bc16_{hh}")
                    nc.vector.tensor_copy(bc16, bc_ps)
                    bcs.append(bc16)
                for mi in range(4):
                    cs = slice(mi * 128, (mi + 1) * 128)
                    gs = slice(jq * 512 + mi * 128, jq * 512 + (mi + 1) * 128)
                    for hh, bc16 in enumerate(bcs):
                        if hh == 0:
                            nc.vector.tensor_mul(oT[t][0:64, gs], o_ps[hh][0:64, cs], bc16[:, cs])
                        else:
                            tmp = znp.tile([64, 128], BT16, tag="tmp_os", bufs=4, name=f"tmf{t}_{jq}_{mi}")
                            nc.vector.tensor_mul(tmp, o_ps[hh][0:64, cs], bc16[:, cs])
                            nc.sync.dma_start(oT[t][64:128, gs], tmp)
                    phase3_m(4 * jq + mi, via_filler=False)

            # state for rationing filler across the 160 attention tile-steps
            steps_total = NP * sum(4 * jq + 4 for jq in range(NJQ))
            step_idx = [0]
            pend = []           # (t, jq, evicted) awaiting normalize_start
            pend_fin = []       # (t, jq, tmps) awaiting normalize_finish

            def future_filler(jq):
                # phase3 units for chunks < 3 not yet enqueued
                return 32 * max(0, min(3, NJQ - 1) - jq)

            def attention_sched(t, jq, last=False):
                nik = 4 * jq + 4
                o_ps = [
                    ps.tile([65, 512], F32, tag="ot", bufs=2, name=f"ops{t}_{jq}_{_h}")
                    for _h in range(2)
                ]
                pts = {}
                for ik in range(nik):
                    d = ik - 4 * jq
                    c0 = 128 * d if d > 0 else 0   # first potentially-valid column
                    st = ps.tile([128, 1024], F32, tag="st", name=f"st{t}_{jq}_{ik}")
                    stg = st.rearrange("p (h q) -> p h q", q=512)
                    for hh in range(2):
                        r = slice(hh * 64, hh * 64 + 64)
                        nc.tensor.matmul(
                            stg[:, hh, c0:512],
                            kT[t][r, ik * 128:(ik + 1) * 128],
                            qT[t][r, jq * 512 + c0:(jq + 1) * 512],
                            start=True, stop=True,
                        )
                    pt = ptp.tile([128, 1024], BT16, tag="pt", name=f"pt{t}_{jq}_{ik}")
                    ptg = pt.rearrange("p (h q) -> p h q", q=512)
                    if d >= 0:
                        ptm = ptmpp.tile([128, 1024], BT16, tag="ptmp", name=f"ptm{t}_{jq}_{ik}")
                        ptmg = ptm.rearrange("p (h q) -> p h q", q=512)
                        nc.scalar.activation(ptmg[:, :, c0:512], stg[:, :, c0:512], AF.Exp)
                        for hh in range(2):
                            nc.vector.tensor_mul(
                                ptg[:, hh, c0:512],
                                ptmg[:, hh, c0:512],
                                masks_sb[:, d, c0:512],
                            )
                    else:
                        nc.scalar.activation(pt, st, AF.Exp)
                    pts[ik] = (pt, c0)
                    # ration filler between S^T and the lagging AV; hold back
                    # for the last chunk so its longer attention rows still
                    # get PE cover while Scalar exps catch up
                    steps_left = steps_total - step_idx[0]
                    avail = len(filler) + future_filler(jq)
                    cap = 2 if jq < NJQ - 1 else 4
                    n = min(cap, max(1, -(-avail // max(1, steps_left))))
                    emit_filler(n)
                    step_idx[0] += 1
                    if ik > 0:
                        av(t, ik - 1, nik, pts, o_ps)
                    if ik == 1 and pend:
                        st_, sjq_, sev_ = pend.pop(0)
                        pend_fin.append((st_, sjq_, normalize_start(st_, sjq_, sev_)))
                    if ik == min(nik - 1, 6) and pend_fin:
                        ft_, fjq_, ftm_ = pend_fin.pop(0)
                        normalize_finish(ft_, fjq_, ftm_)
                        if ft_ == NP - 1 and fjq_ < NJQ - 1:
                            enqueue_phase3(fjq_)
                av(t, nik - 1, nik, pts, o_ps)
                if last:
                    # tail pair: normalize_fast reads O^T/Z straight from PSUM
                    pend.append((t, jq, o_ps))
                    return
                # evict Z row + unnormalized O^T, freeing the PSUM accumulators
                out_h = []
                for hh in range(2):
                    ouz = znp.tile([65, 512], F32, tag="ouz", bufs=6, name=f"oz{t}_{jq}_{hh}")
                    nc.vector.tensor_copy(ouz, o_ps[hh])
                    out_h.append(ouz)
                pend.append((t, jq, out_h))

            # ================= main schedule =================
            # upfront: phase 1 chunk 0, k-major with 4 simultaneously-open
            # PSUM groups (2 "fp" + 2 borrowed "ot" slots) so PE consumption
            # tracks the HBM-bound weight stream chunk by chunk instead of
            # each group stalling on the full tensor
            def boot_ps(idx, nm):
                return ps.tile([128, 512], F32, tag=("fp" if idx < 2 else "ot"),
                               name=nm)

            bq_ps = [boot_ps(t, f"boot_q{t}") for t in range(NP)]
            for k in range(KC):
                for t in range(NP):
                    nc.tensor.matmul(
                        bq_ps[t], wq_sb[:, k, t * 128:(t + 1) * 128], xt0[:, k, :],
                        start=(k == 0), stop=(k == KC - 1),
                    )
            for t in range(NP):
                nc.scalar.activation(
                    qT[t][:, 0:512], bq_ps[t],
                    AF.Identity, bias=bq_sb[:, t:t + 1], scale=0.125,
                )
            bk_ps = [boot_ps(t, f"boot_k{t}") for t in range(NP)]
            for k in range(KC):
                for t in range(NP):
                    nc.tensor.matmul(
                        bk_ps[t], wk_sb[:, k, t * 128:(t + 1) * 128], xt0[:, k, :],
                        start=(k == 0), stop=(k == KC - 1),
                    )
            for t in range(NP):
                nc.scalar.activation(
                    kT[t][:, 0:512], bk_ps[t],
                    AF.Identity, bias=bk_sb[:, t:t + 1], scale=1.0,
                )
            bv_ps = [boot_ps(s, f"boot_v{s}") for s in range(4)]
            for k in range(KC):
                for s in range(4):
                    nc.tensor.matmul(
                        bv_ps[s], xt0[:, k, s * 128:(s + 1) * 128], wv_sb[:, k, :],
                        start=(k == 0), stop=(k == KC - 1),
                    )
            for s in range(4):
                vg = v_sb[s].rearrange("p (h c) -> p h c", c=65)
                nc.vector.tensor_copy(
                    vg[:, :, 0:64], bv_ps[s].rearrange("p (h c) -> p h c", c=64)
                )
            # wo is first needed by phase3 at ~t+100us; issue it only now so
            # its 2MB doesn't compete with the startup-critical stream
            nc.gpsimd.dma_start(wo_sb, wo.rearrange("(t p) c -> p t c", p=128))
            # prefetch DMA + enqueue filler for later chunks
            xt_dma(1)
            enqueue_phase1(1)
            for jq in range(NJQ):
                if jq + 2 < NJQ:
                    xt_dma(jq + 2)
                    enqueue_phase1(jq + 2)
                for t in range(NP):
                    attention_sched(t, jq, last=(jq == NJQ - 1 and t == NP - 1))
                if jq + 1 < NJQ:
                    drain_to(p1_watermark[jq + 1])
            # tail: flush stragglers, then the last pair interleaves its
            # normalization with phase3 m-chunks (jq=3's phase3 runs here).
            while len(filler):
                emit_filler(len(filler))
            for ft_, fjq_, ftm_ in pend_fin:
                normalize_finish(ft_, fjq_, ftm_)
                if ft_ == NP - 1 and fjq_ < NJQ - 1:
                    enqueue_phase3(fjq_)
                    emit_filler(len(filler))
            for st_, sjq_, sev_ in pend[:-1]:
                tm_ = normalize_start(st_, sjq_, sev_)
                normalize_finish(st_, sjq_, tm_)
                if st_ == NP - 1 and sjq_ < NJQ - 1:
                    enqueue_phase3(sjq_)
                    emit_filler(len(filler))
            pt_, pjq_, pev_ = pend[-1]
            normalize_fast(pt_, pjq_, pev_)

    nc.compile()
    return nc


def _host_prep(x, wq, bq, wk, bk, wv, wo):
    masks_np = np.zeros((128, 4, 512), dtype=BF16)
    qn = np.arange(512)[None, :]
    kn = np.arange(128)[:, None]
    for d in range(4):
        masks_np[:, d, :] = (qn >= kn + 128 * d).astype(BF16)

    per_g = []
    for g in range(G):
        cs = slice(g * HD, (g + 1) * HD)
        per_g.append({
            "wq": np.ascontiguousarray(wq[:, cs]).astype(BF16),
            "wk": np.ascontiguousarray(wk[:, cs]).astype(BF16),
            "wv": np.ascontiguousarray(wv[:, cs]).astype(BF16),
            "wo": np.ascontiguousarray(wo[cs, :]).astype(BF16),
            "bq": np.ascontiguousarray((bq[cs] / 8.0).reshape(NP, 128).T).astype(np.float32),
            "bk": np.ascontiguousarray(bk[cs].reshape(NP, 128).T).astype(np.float32),
            "masks": masks_np,
        })
    in_maps = []
    for c in range(8):
        b, g = divmod(c, G)
        m = dict(per_g[g])
        m["xT"] = np.ascontiguousarray(x[b].T).astype(BF16)
        in_maps.append(m)
    return in_maps


def kernel(x, wq, bq, wk, bk, wv, bv, wo, bo):
    x = np.asarray(x, dtype=np.float32)
    wq = np.asarray(wq, dtype=np.float32)
    bq = np.asarray(bq, dtype=np.float32)
    wk = np.asarray(wk, dtype=np.float32)
    bk = np.asarray(bk, dtype=np.float32)
    wv = np.asarray(wv, dtype=np.float32)
    bv = np.asarray(bv, dtype=np.float32)
    wo = np.asarray(wo, dtype=np.float32)
    bo = np.asarray(bo, dtype=np.float32)

    if "nc" not in _CACHED:
        _CACHED["nc"] = _build()
    nc = _CACHED["nc"]

    in_maps = _host_prep(x, wq, bq, wk, bk, wv, wo)
    res = run_bass_kernel_spmd(nc, in_maps, core_ids=list(range(8)))

    const_row = (bo.astype(np.float64) + bv.astype(np.float64) @ wo.astype(np.float64))
    out = np.empty((B, T, C), dtype=np.float32)
    for b in range(B):
        # y is stored n-major [2, T, 512]; reassemble the C axis
        y0 = np.asarray(res.results[2 * b]["y"], dtype=np.float64)
        y1 = np.asarray(res.results[2 * b + 1]["y"], dtype=np.float64)
        acc = np.concatenate([y0[0], y0[1]], axis=-1)
        acc += np.concatenate([y1[0], y1[1]], axis=-1)
        acc += const_row[None, :]
        out[b] = acc.astype(np.float32)
    return out
